# revision 18
# baseline (speedup 1.0000x reference)
"""Trainium2 Bass kernel for nn_Former_Mobile (mobile-former style cross-attention).

Computation (per batch item n):
    kv   = relu6(global_feature @ W_kv^T + b_kv)        # [m=8, 2c]
    K, V = kv[:, :c], kv[:, c:]                         # [8, c=384]
    q    = x reshaped [hw=3136, c]
    attn = softmax(q @ K^T)                             # [hw, 8]
    out  = (attn @ V) reshaped back + x                 # [c, hw]

Sharding: data-parallel over batch n across 8 NeuronCores (4 items each);
W_kv/b_kv replicated (bias folded in as a K=1 contraction row host-side).

The kernel is HBM-bound (~41MB in+out per core); engine placement:
  PE    : kv projection (one [32,768] series), mm1 scoresT (KT stationary
          M=32, x streaming, f32r, kc-outer so compute starts on the first
          x chunk; 7 subtiles packed 4-per-psum-bank on partition
          quarters), mm2 outT (V stationary K=8, attnT bf16 streaming),
          identity-matmul residual accumulate for part of the tiles.
  DVE   : 32x32 stream-transposes fold scoresT psum tiles into
          SC[p, 32j+c] = scores[q=32j+p, kvrow=c]; softmax along the free
          dim on c-slice n*8..n*8+8 writes bf16 attn redirected to c 0..8;
          a second stream-transpose yields attnT[8, hw] bf16 on partitions
          0-7 for mm2. Plus residual adds for t7<4.
  ACT   : exp, psum drains for the PE-residual tiles, output dma issues.
  Pool  : softmax subtract.
  Sync  : input dma issues.
mm1 operands are float32r (1 col/cycle at N>=256, fp32 psum accumulation);
softmax math is fp32; attn/V are bf16 (post-softmax, well within the
error budget); residual/store are fp32.
"""

import sys

if "/opt/trn_rl_repo" not in sys.path:
    sys.path.insert(0, "/opt/trn_rl_repo")

import numpy as np

N, C, H, W = 32, 384, 56, 56
HW = H * W                      # 3136
M, D = 8, 768
N_CORES = 8
N_LOC = N // N_CORES            # 4 batch items per core
NM = N_LOC * M                  # 32 kv rows per core
D1 = D + 1                      # 768 + bias row
KC = C // 128                   # 3 contraction chunks over c
P = 128
T5 = 448                        # mm1/mm2 free-dim tile (7 per hw row)
NT5 = HW // T5                  # 7
NJ = HW // 32                   # 98 folded blocks

_cache = {}
last_results = None


def _build():
    from concourse import bacc, tile, mybir
    from concourse.masks import make_identity

    f32 = mybir.dt.float32
    f32r = mybir.dt.float32r
    bf16 = mybir.dt.bfloat16
    fp16 = mybir.dt.float16
    Alu = mybir.AluOpType
    Act = mybir.ActivationFunctionType
    PSUM = tile.bass.MemorySpace.PSUM

    nc = bacc.Bacc("TRN2", target_bir_lowering=False, debug=False,
                   num_devices=N_CORES)

    xs_d = nc.dram_tensor("xs", [N_LOC, C, HW], fp16, kind="ExternalInput")
    gft_d = nc.dram_tensor("gft", [D1, NM], f32r, kind="ExternalInput")
    wt_d = nc.dram_tensor("wt", [D1, D], f32r, kind="ExternalInput")
    out_d = nc.dram_tensor("out", [N_LOC, C, HW], f32, kind="ExternalOutput")

    with tile.TileContext(nc) as tc:
        with tc.tile_pool(name="const", bufs=1) as const:
            ident = const.tile([P, P], f32, tag="ident")
            make_identity(nc, ident[:, :])
            identr = const.tile([P, P], f32r, tag="identr")
            nc.vector.tensor_copy(identr[:, :], ident[:, :])
            identb = const.tile([P, P], fp16, tag="identb")
            nc.vector.tensor_copy(identb[:, :], ident[:, :])

            K_sb = const.tile([NM, C], f32r, tag="K_sb")
            V_sb = const.tile([NM, C], bf16, tag="V_sb")
            # per-item V rows at partition 0 (engine APs can't start at
            # partition 8/16/24); item 0 reads V_sb[0:8] directly
            V_n = [const.tile([M, C], bf16, tag=f"V{n}", name=f"V{n}")
                   for n in range(1, N_LOC)]
            KT = [const.tile([P, NM], fp16, tag=f"KT{kc}", name=f"KT{kc}")
                  for kc in range(KC)]

            with tc.tile_pool(name="wtp", bufs=1) as wtp, \
                 tc.tile_pool(name="psum0", bufs=1, space=PSUM) as psum0:
                wt_sb = []
                gft_sb = []
                for i in range(7):
                    rows = P if i < 6 else 1
                    g = wtp.tile([rows, NM], f32r, tag=f"gft{i}",
                                 name=f"gft{i}")
                    nc.sync.dma_start(g[:, :],
                                      gft_d.ap()[i * P:i * P + rows, :])
                    gft_sb.append(g)
                for i in range(7):
                    rows = P if i < 6 else 1
                    w = wtp.tile([rows, D], f32r, tag=f"wt{i}", name=f"wt{i}")
                    nc.sync.dma_start(w[:, :], wt_d.ap()[i * P:i * P + rows, :])
                    wt_sb.append(w)
                kvK = psum0.tile([NM, C], f32, tag="kvK")
                kvV = psum0.tile([NM, C], f32, tag="kvV")
                for i in range(7):
                    nc.tensor.matmul(
                        kvK[:, :], gft_sb[i][:, :], wt_sb[i][:, :C],
                        start=(i == 0), stop=(i == 6))
                for i in range(7):
                    nc.tensor.matmul(
                        kvV[:, :], gft_sb[i][:, :], wt_sb[i][:, C:],
                        start=(i == 0), stop=(i == 6))
                nc.vector.tensor_scalar(K_sb[:, :], kvK[:, :], 0.0, 6.0,
                                        op0=Alu.max, op1=Alu.min)
                nc.vector.tensor_scalar(V_sb[:, :], kvV[:, :], 0.0, 6.0,
                                        op0=Alu.max, op1=Alu.min)
                for n in range(1, N_LOC):
                    nc.sync.dma_start(V_n[n - 1][:, :],
                                      V_sb[n * M:(n + 1) * M, :])
                for kc in range(KC):
                    ktp = psum0.tile([P, NM], f32r, tag="ktp")
                    nc.tensor.transpose(ktp[:, :],
                                        K_sb[:, kc * P:(kc + 1) * P],
                                        identr[:NM, :NM])
                    nc.scalar.copy(KT[kc][:, :], ktp[:, :])

            with (
                tc.tile_pool(name="xp", bufs=12) as xp,
                tc.tile_pool(name="scf", bufs=2) as scfp,
                tc.tile_pool(name="abp", bufs=2) as abp,
                tc.tile_pool(name="sm", bufs=4) as sm,
                tc.tile_pool(name="aTp", bufs=2) as aTpool,
                tc.tile_pool(name="op", bufs=2) as op,
                tc.tile_pool(name="p32", bufs=1, space=PSUM) as p32,
                tc.tile_pool(name="ps_o", bufs=5, space=PSUM) as ps_o,
            ):
                def load_x(n):
                    xc = []
                    for kc in range(KC):
                        t = xp.tile([P, HW], fp16, tag="x", name="x")
                        nc.sync.dma_start(
                            t[:, :], xs_d.ap()[n, kc * P:(kc + 1) * P, :])
                        xc.append(t)
                    return xc

                def gen_out(n, aT, xc):
                    # mm2 + residual + store for item n; one step per yield
                    # so it interleaves with the next item's attention
                    # phase. Residual is split: t7<4 adds psum+x on DVE;
                    # t7>=4 accumulates x into psum on the PE (identity
                    # stationary) and ACT drains the finished tile.
                    for kc in range(KC):
                        osb = op.tile([P, HW], f32, tag="o", name="osb")
                        for t7 in range(NT5):
                            sl = slice(t7 * T5, (t7 + 1) * T5)
                            po = ps_o.tile([P, T5], f32, tag="po", name="po")
                            pe_res = t7 == 6
                            nc.tensor.matmul(
                                po[:, :],
                                (V_sb[0:M, kc * P:(kc + 1) * P] if n == 0
                                 else V_n[n - 1][:, kc * P:(kc + 1) * P]),
                                aT[0:M, sl],
                                start=True, stop=not pe_res)
                            if pe_res:
                                nc.tensor.matmul(
                                    po[:, :], identb[:, :], xc[kc][:, sl],
                                    start=False, stop=True)
                                nc.scalar.copy(osb[:, sl], po[:, :])
                            elif t7 >= 4:
                                nc.scalar.copy(osb[:, sl], po[:, :])
                                nc.gpsimd.tensor_add(
                                    osb[:, sl], osb[:, sl], xc[kc][:, sl])
                            else:
                                nc.vector.tensor_add(
                                    osb[:, sl], po[:, :],
                                    xc[kc][:, sl])
                            yield
                        nc.scalar.dma_start(
                            out_d.ap()[n, kc * P:(kc + 1) * P, :],
                            osb[:, :])
                        yield

                def drain(gen, steps):
                    if gen is None:
                        return None
                    try:
                        for _ in range(steps):
                            next(gen)
                    except StopIteration:
                        return None
                    return gen

                outgen = None
                xtiles = [None] * N_LOC
                xtiles[0] = load_x(0)
                xtiles[1] = load_x(1)
                xtiles[2] = load_x(2)
                for n in range(N_LOC):
                    xc = xtiles[n]

                    # mm1: scoresT [32, hw] with kc as the OUTER loop so
                    # the first matmul sweep starts as soon as x chunk 0
                    # lands. The 7 hw-subtiles pack 3-per-psum-bank on
                    # partition thirds (bf16 matmuls may target base
                    # partition 0/32/64; quadrant 3 is unusable).
                    SC = scfp.tile([NM, HW], f32, tag="scf")
                    pst = [p32.tile([P, T5], f32, tag=f"pst{b}",
                                    name=f"pst{b}") for b in range(3)]
                    for kc in range(KC):
                        for t5 in range(NT5):
                            q = t5 % 3
                            nc.tensor.matmul(
                                pst[t5 // 3][q * NM:(q + 1) * NM, :],
                                KT[kc][:, :],
                                xc[kc][:, t5 * T5:(t5 + 1) * T5],
                                start=(kc == 0), stop=(kc == KC - 1))
                            if kc > 0:
                                outgen = drain(outgen, 2)
                    for t5 in range(NT5):
                        q = t5 % 3
                        nc.vector.transpose(
                            SC[:, t5 * T5:(t5 + 1) * T5],
                            pst[t5 // 3][q * NM:(q + 1) * NM, :])
                        outgen = drain(outgen, 1)

                    # softmax along free dim in the folded layout:
                    # SC[p, 32j+c] = scores[q=32j+p, kvrow=c]; item n's
                    # scores live at c = n*8..n*8+8. attn is written back
                    # redirected to c = 0..8 so the unfold lands attnT on
                    # partitions 0-7.
                    sc3 = SC[:, :].rearrange("p (j c) -> p j c", c=32)
                    ssl = sc3[:, :, n * M:(n + 1) * M]
                    nmx = sm.tile([NM, NJ], f32, tag="nmx")
                    nc.vector.tensor_reduce(nmx[:, :], ssl,
                                            axis=mybir.AxisListType.X,
                                            op=Alu.max, negate=True)
                    nmx_b = nmx[:, :].unsqueeze(-1).broadcast_to([NM, NJ, M])
                    nc.gpsimd.tensor_add(ssl, ssl, nmx_b)
                    outgen = drain(outgen, 2)
                    nc.scalar.activation(ssl, ssl, Act.Exp)
                    den = sm.tile([NM, NJ], f32, tag="den")
                    nc.vector.tensor_reduce(den[:, :], ssl,
                                            axis=mybir.AxisListType.X,
                                            op=Alu.add)
                    r = sm.tile([NM, NJ], f32, tag="r")
                    nc.vector.reciprocal(r[:, :], den[:, :])
                    r_b = r[:, :].unsqueeze(-1).broadcast_to([NM, NJ, M])
                    # attn lands in bf16 (2-byte transposes; no f32r
                    # rounding rule), redirected to c 0..8. Cols 8..31 of
                    # each block stay unwritten; the unfold moves those
                    # bits into rows 8..31 of aT, which nothing reads.
                    AB = abp.tile([NM, HW], bf16, tag="ab")
                    ab3 = AB[:, :].rearrange("p (j c) -> p j c", c=32)
                    nc.vector.tensor_mul(ab3[:, :, 0:M], ssl, r_b)
                    outgen = drain(outgen, 2)

                    # unfold: attnT [8, hw] on partitions 0-7
                    aT = aTpool.tile([NM, HW], bf16, tag="aT")
                    hh = HW // 2
                    nc.vector.transpose(aT[:, :hh], AB[:, :hh])
                    nc.vector.transpose(aT[:, hh:], AB[:, hh:])

                    # flush the previous item's output phase, then queue ours
                    while outgen is not None:
                        outgen = drain(outgen, 4)
                    outgen = gen_out(n, aT, xc)
                    if n + 3 < N_LOC:
                        xtiles[n + 3] = load_x(n + 3)
                while outgen is not None:
                    outgen = drain(outgen, 4)

    nc.compile()
    return nc


def get_nc():
    if "nc" not in _cache:
        _cache["nc"] = _build()
    return _cache["nc"]


def make_in_maps(x, global_feature, W_kv, b_kv):
    x = np.ascontiguousarray(
        np.asarray(x, np.float32).reshape(N, C, HW).astype(np.float16))
    wt = np.zeros((D1, D), np.float32)
    wt[:D] = np.asarray(W_kv, np.float32).T
    wt[D] = np.asarray(b_kv, np.float32)
    gf = np.asarray(global_feature, np.float32)
    in_maps = []
    for i in range(N_CORES):
        gfl = gf[i * N_LOC:(i + 1) * N_LOC].reshape(NM, D)
        gft = np.zeros((D1, NM), np.float32)
        gft[:D] = gfl.T
        gft[D] = 1.0
        in_maps.append({
            "xs": np.ascontiguousarray(x[i * N_LOC:(i + 1) * N_LOC]),
            "gft": gft,
            "wt": wt,
        })
    return in_maps


def kernel(x, global_feature, W_kv, b_kv, trace=False):
    global last_results
    from concourse.bass_utils import run_bass_kernel_spmd

    nc = get_nc()
    in_maps = make_in_maps(x, global_feature, W_kv, b_kv)
    res = run_bass_kernel_spmd(nc, in_maps, core_ids=list(range(N_CORES)),
                               trace=trace)
    last_results = res
    out = np.concatenate([res.results[i]["out"][None] for i in range(N_CORES)],
                         axis=0)
    return out.reshape(N, C, H, W).astype(np.float32)


# revision 19
# speedup vs baseline: 1.0039x; 1.0039x over previous
"""Trainium2 Bass kernel for nn_Former_Mobile (mobile-former style cross-attention).

Computation (per batch item n):
    kv   = relu6(global_feature @ W_kv^T + b_kv)        # [m=8, 2c]
    K, V = kv[:, :c], kv[:, c:]                         # [8, c=384]
    q    = x reshaped [hw=3136, c]
    attn = softmax(q @ K^T)                             # [hw, 8]
    out  = (attn @ V) reshaped back + x                 # [c, hw]

Sharding: data-parallel over batch n across 8 NeuronCores (4 items each);
W_kv/b_kv replicated (bias folded in as a K=1 contraction row host-side).

The kernel is HBM-bound (~41MB in+out per core); engine placement:
  PE    : kv projection (one [32,768] series), mm1 scoresT (KT stationary
          M=32, x streaming, f32r, kc-outer so compute starts on the first
          x chunk; 7 subtiles packed 4-per-psum-bank on partition
          quarters), mm2 outT (V stationary K=8, attnT bf16 streaming),
          identity-matmul residual accumulate for part of the tiles.
  DVE   : 32x32 stream-transposes fold scoresT psum tiles into
          SC[p, 32j+c] = scores[q=32j+p, kvrow=c]; softmax along the free
          dim on c-slice n*8..n*8+8 writes bf16 attn redirected to c 0..8;
          a second stream-transpose yields attnT[8, hw] bf16 on partitions
          0-7 for mm2. Plus residual adds for t7<4.
  ACT   : exp, psum drains for the PE-residual tiles, output dma issues.
  Pool  : softmax subtract.
  Sync  : input dma issues.
mm1 operands are float32r (1 col/cycle at N>=256, fp32 psum accumulation);
softmax math is fp32; attn/V are bf16 (post-softmax, well within the
error budget); residual/store are fp32.
"""

import sys

if "/opt/trn_rl_repo" not in sys.path:
    sys.path.insert(0, "/opt/trn_rl_repo")

import numpy as np

N, C, H, W = 32, 384, 56, 56
HW = H * W                      # 3136
M, D = 8, 768
N_CORES = 8
N_LOC = N // N_CORES            # 4 batch items per core
NM = N_LOC * M                  # 32 kv rows per core
D1 = D + 1                      # 768 + bias row
KC = C // 128                   # 3 contraction chunks over c
P = 128
T5 = 448                        # mm1/mm2 free-dim tile (7 per hw row)
NT5 = HW // T5                  # 7
NJ = HW // 32                   # 98 folded blocks

_cache = {}
last_results = None


def _build():
    from concourse import bacc, tile, mybir
    from concourse.masks import make_identity

    f32 = mybir.dt.float32
    f32r = mybir.dt.float32r
    bf16 = mybir.dt.bfloat16
    fp16 = mybir.dt.float16
    Alu = mybir.AluOpType
    Act = mybir.ActivationFunctionType
    PSUM = tile.bass.MemorySpace.PSUM

    nc = bacc.Bacc("TRN2", target_bir_lowering=False, debug=False,
                   num_devices=N_CORES)

    xs_d = nc.dram_tensor("xs", [N_LOC, C, HW], fp16, kind="ExternalInput")
    gft_d = nc.dram_tensor("gft", [D1, NM], f32r, kind="ExternalInput")
    wt_d = nc.dram_tensor("wt", [D1, D], f32r, kind="ExternalInput")
    out_d = nc.dram_tensor("out", [N_LOC, C, HW], f32, kind="ExternalOutput")

    with tile.TileContext(nc) as tc:
        with tc.tile_pool(name="const", bufs=1) as const:
            ident = const.tile([P, P], f32, tag="ident")
            make_identity(nc, ident[:, :])
            identr = const.tile([P, P], f32r, tag="identr")
            nc.vector.tensor_copy(identr[:, :], ident[:, :])
            identb = const.tile([P, P], fp16, tag="identb")
            nc.vector.tensor_copy(identb[:, :], ident[:, :])

            K_sb = const.tile([NM, C], f32r, tag="K_sb")
            V_sb = const.tile([NM, C], bf16, tag="V_sb")
            # per-item V rows at partition 0 (engine APs can't start at
            # partition 8/16/24); item 0 reads V_sb[0:8] directly
            V_n = [const.tile([M, C], bf16, tag=f"V{n}", name=f"V{n}")
                   for n in range(1, N_LOC)]
            KT = [const.tile([P, NM], fp16, tag=f"KT{kc}", name=f"KT{kc}")
                  for kc in range(KC)]

            with tc.tile_pool(name="wtp", bufs=1) as wtp, \
                 tc.tile_pool(name="psum0", bufs=1, space=PSUM) as psum0:
                wt_sb = []
                gft_sb = []
                for i in range(7):
                    rows = P if i < 6 else 1
                    g = wtp.tile([rows, NM], f32r, tag=f"gft{i}",
                                 name=f"gft{i}")
                    nc.sync.dma_start(g[:, :],
                                      gft_d.ap()[i * P:i * P + rows, :])
                    gft_sb.append(g)
                for i in range(7):
                    rows = P if i < 6 else 1
                    w = wtp.tile([rows, D], f32r, tag=f"wt{i}", name=f"wt{i}")
                    nc.sync.dma_start(w[:, :], wt_d.ap()[i * P:i * P + rows, :])
                    wt_sb.append(w)
                kvK = psum0.tile([NM, C], f32, tag="kvK")
                kvV = psum0.tile([NM, C], f32, tag="kvV")
                for i in range(7):
                    nc.tensor.matmul(
                        kvK[:, :], gft_sb[i][:, :], wt_sb[i][:, :C],
                        start=(i == 0), stop=(i == 6))
                for i in range(7):
                    nc.tensor.matmul(
                        kvV[:, :], gft_sb[i][:, :], wt_sb[i][:, C:],
                        start=(i == 0), stop=(i == 6))
                nc.vector.tensor_scalar(K_sb[:, :], kvK[:, :], 0.0, 6.0,
                                        op0=Alu.max, op1=Alu.min)
                nc.vector.tensor_scalar(V_sb[:, :], kvV[:, :], 0.0, 6.0,
                                        op0=Alu.max, op1=Alu.min)
                for n in range(1, N_LOC):
                    nc.sync.dma_start(V_n[n - 1][:, :],
                                      V_sb[n * M:(n + 1) * M, :])
                for kc in range(KC):
                    ktp = psum0.tile([P, NM], f32r, tag="ktp")
                    nc.tensor.transpose(ktp[:, :],
                                        K_sb[:, kc * P:(kc + 1) * P],
                                        identr[:NM, :NM])
                    nc.scalar.copy(KT[kc][:, :], ktp[:, :])

            with (
                tc.tile_pool(name="xp", bufs=12) as xp,
                tc.tile_pool(name="scf", bufs=2) as scfp,
                tc.tile_pool(name="abp", bufs=2) as abp,
                tc.tile_pool(name="sm", bufs=4) as sm,
                tc.tile_pool(name="aTp", bufs=2) as aTpool,
                tc.tile_pool(name="op", bufs=2) as op,
                tc.tile_pool(name="p32", bufs=1, space=PSUM) as p32,
                tc.tile_pool(name="ps_o", bufs=5, space=PSUM) as ps_o,
            ):
                def load_x(n):
                    xc = []
                    for kc in range(KC):
                        t = xp.tile([P, HW], fp16, tag="x", name="x")
                        nc.sync.dma_start(
                            t[:, :], xs_d.ap()[n, kc * P:(kc + 1) * P, :])
                        xc.append(t)
                    return xc

                def gen_out(n, aT, xc):
                    # mm2 + residual + store for item n; one step per yield
                    # so it interleaves with the next item's attention
                    # phase. Residual is split: t7<4 adds psum+x on DVE;
                    # t7>=4 accumulates x into psum on the PE (identity
                    # stationary) and ACT drains the finished tile.
                    for kc in range(KC):
                        osb = op.tile([P, HW], f32, tag="o", name="osb")
                        for t7 in range(NT5):
                            sl = slice(t7 * T5, (t7 + 1) * T5)
                            po = ps_o.tile([P, T5], f32, tag="po", name="po")
                            pe_res = t7 >= 4
                            nc.tensor.matmul(
                                po[:, :],
                                (V_sb[0:M, kc * P:(kc + 1) * P] if n == 0
                                 else V_n[n - 1][:, kc * P:(kc + 1) * P]),
                                aT[0:M, sl],
                                start=True, stop=not pe_res)
                            if pe_res:
                                nc.tensor.matmul(
                                    po[:, :], identb[:, :], xc[kc][:, sl],
                                    start=False, stop=True)
                                nc.scalar.copy(osb[:, sl], po[:, :])
                            else:
                                nc.vector.tensor_add(
                                    osb[:, sl], po[:, :],
                                    xc[kc][:, sl])
                            yield
                        nc.scalar.dma_start(
                            out_d.ap()[n, kc * P:(kc + 1) * P, :],
                            osb[:, :])
                        yield

                def drain(gen, steps):
                    if gen is None:
                        return None
                    try:
                        for _ in range(steps):
                            next(gen)
                    except StopIteration:
                        return None
                    return gen

                outgen = None
                xtiles = [None] * N_LOC
                xtiles[0] = load_x(0)
                xtiles[1] = load_x(1)
                xtiles[2] = load_x(2)
                for n in range(N_LOC):
                    xc = xtiles[n]

                    # mm1: scoresT [32, hw] with kc as the OUTER loop so
                    # the first matmul sweep starts as soon as x chunk 0
                    # lands. The 7 hw-subtiles pack 3-per-psum-bank on
                    # partition thirds (bf16 matmuls may target base
                    # partition 0/32/64; quadrant 3 is unusable).
                    SC = scfp.tile([NM, HW], f32, tag="scf")
                    pst = [p32.tile([P, T5], f32, tag=f"pst{b}",
                                    name=f"pst{b}") for b in range(3)]
                    for kc in range(KC):
                        for t5 in range(NT5):
                            q = t5 % 3
                            nc.tensor.matmul(
                                pst[t5 // 3][q * NM:(q + 1) * NM, :],
                                KT[kc][:, :],
                                xc[kc][:, t5 * T5:(t5 + 1) * T5],
                                start=(kc == 0), stop=(kc == KC - 1))
                            if kc > 0:
                                outgen = drain(outgen, 2)
                    for t5 in range(NT5):
                        q = t5 % 3
                        nc.vector.transpose(
                            SC[:, t5 * T5:(t5 + 1) * T5],
                            pst[t5 // 3][q * NM:(q + 1) * NM, :])
                        outgen = drain(outgen, 1)

                    # softmax along free dim in the folded layout:
                    # SC[p, 32j+c] = scores[q=32j+p, kvrow=c]; item n's
                    # scores live at c = n*8..n*8+8. attn is written back
                    # redirected to c = 0..8 so the unfold lands attnT on
                    # partitions 0-7.
                    sc3 = SC[:, :].rearrange("p (j c) -> p j c", c=32)
                    ssl = sc3[:, :, n * M:(n + 1) * M]
                    nmx = sm.tile([NM, NJ], f32, tag="nmx")
                    nc.vector.tensor_reduce(nmx[:, :], ssl,
                                            axis=mybir.AxisListType.X,
                                            op=Alu.max, negate=True)
                    nmx_b = nmx[:, :].unsqueeze(-1).broadcast_to([NM, NJ, M])
                    nc.gpsimd.tensor_add(ssl, ssl, nmx_b)
                    outgen = drain(outgen, 2)
                    nc.scalar.activation(ssl, ssl, Act.Exp)
                    den = sm.tile([NM, NJ], f32, tag="den")
                    nc.vector.tensor_reduce(den[:, :], ssl,
                                            axis=mybir.AxisListType.X,
                                            op=Alu.add)
                    r = sm.tile([NM, NJ], f32, tag="r")
                    nc.vector.reciprocal(r[:, :], den[:, :])
                    r_b = r[:, :].unsqueeze(-1).broadcast_to([NM, NJ, M])
                    # attn lands in bf16 (2-byte transposes; no f32r
                    # rounding rule), redirected to c 0..8. Cols 8..31 of
                    # each block stay unwritten; the unfold moves those
                    # bits into rows 8..31 of aT, which nothing reads.
                    AB = abp.tile([NM, HW], bf16, tag="ab")
                    ab3 = AB[:, :].rearrange("p (j c) -> p j c", c=32)
                    nc.vector.tensor_mul(ab3[:, :, 0:M], ssl, r_b)
                    outgen = drain(outgen, 2)

                    # unfold: attnT [8, hw] on partitions 0-7
                    aT = aTpool.tile([NM, HW], bf16, tag="aT")
                    hh = HW // 2
                    nc.vector.transpose(aT[:, :hh], AB[:, :hh])
                    nc.vector.transpose(aT[:, hh:], AB[:, hh:])

                    # flush the previous item's output phase, then queue ours
                    while outgen is not None:
                        outgen = drain(outgen, 4)
                    outgen = gen_out(n, aT, xc)
                    if n + 3 < N_LOC:
                        xtiles[n + 3] = load_x(n + 3)
                while outgen is not None:
                    outgen = drain(outgen, 4)

    nc.compile()
    return nc


def get_nc():
    if "nc" not in _cache:
        _cache["nc"] = _build()
    return _cache["nc"]


def make_in_maps(x, global_feature, W_kv, b_kv):
    x = np.ascontiguousarray(
        np.asarray(x, np.float32).reshape(N, C, HW).astype(np.float16))
    wt = np.zeros((D1, D), np.float32)
    wt[:D] = np.asarray(W_kv, np.float32).T
    wt[D] = np.asarray(b_kv, np.float32)
    gf = np.asarray(global_feature, np.float32)
    in_maps = []
    for i in range(N_CORES):
        gfl = gf[i * N_LOC:(i + 1) * N_LOC].reshape(NM, D)
        gft = np.zeros((D1, NM), np.float32)
        gft[:D] = gfl.T
        gft[D] = 1.0
        in_maps.append({
            "xs": np.ascontiguousarray(x[i * N_LOC:(i + 1) * N_LOC]),
            "gft": gft,
            "wt": wt,
        })
    return in_maps


def kernel(x, global_feature, W_kv, b_kv, trace=False):
    global last_results
    from concourse.bass_utils import run_bass_kernel_spmd

    nc = get_nc()
    in_maps = make_in_maps(x, global_feature, W_kv, b_kv)
    res = run_bass_kernel_spmd(nc, in_maps, core_ids=list(range(N_CORES)),
                               trace=trace)
    last_results = res
    out = np.concatenate([res.results[i]["out"][None] for i in range(N_CORES)],
                         axis=0)
    return out.reshape(N, C, H, W).astype(np.float32)


# revision 20
# speedup vs baseline: 1.0123x; 1.0084x over previous
"""Trainium2 Bass kernel for nn_Former_Mobile (mobile-former style cross-attention).

Computation (per batch item n):
    kv   = relu6(global_feature @ W_kv^T + b_kv)        # [m=8, 2c]
    K, V = kv[:, :c], kv[:, c:]                         # [8, c=384]
    q    = x reshaped [hw=3136, c]
    attn = softmax(q @ K^T)                             # [hw, 8]
    out  = (attn @ V) reshaped back + x                 # [c, hw]

Sharding: data-parallel over batch n across 8 NeuronCores (4 items each);
W_kv/b_kv replicated (bias folded in as a K=1 contraction row host-side).

The kernel is HBM-bound (~41MB in+out per core); engine placement:
  PE    : kv projection (one [32,768] series), mm1 scoresT (KT stationary
          M=32, x streaming, f32r, kc-outer so compute starts on the first
          x chunk; 7 subtiles packed 4-per-psum-bank on partition
          quarters), mm2 outT (V stationary K=8, attnT bf16 streaming),
          identity-matmul residual accumulate for part of the tiles.
  DVE   : 32x32 stream-transposes fold scoresT psum tiles into
          SC[p, 32j+c] = scores[q=32j+p, kvrow=c]; softmax along the free
          dim on c-slice n*8..n*8+8 writes bf16 attn redirected to c 0..8;
          a second stream-transpose yields attnT[8, hw] bf16 on partitions
          0-7 for mm2. Plus residual adds for t7<4.
  ACT   : exp, psum drains for the PE-residual tiles, output dma issues.
  Pool  : softmax subtract.
  Sync  : input dma issues.
mm1 operands are float32r (1 col/cycle at N>=256, fp32 psum accumulation);
softmax math is fp32; attn/V are bf16 (post-softmax, well within the
error budget); residual/store are fp32.
"""

import sys

if "/opt/trn_rl_repo" not in sys.path:
    sys.path.insert(0, "/opt/trn_rl_repo")

import numpy as np

N, C, H, W = 32, 384, 56, 56
HW = H * W                      # 3136
M, D = 8, 768
N_CORES = 8
N_LOC = N // N_CORES            # 4 batch items per core
NM = N_LOC * M                  # 32 kv rows per core
D1 = D + 1                      # 768 + bias row
KC = C // 128                   # 3 contraction chunks over c
P = 128
T5 = 448                        # mm1/mm2 free-dim tile (7 per hw row)
NT5 = HW // T5                  # 7
NJ = HW // 32                   # 98 folded blocks

_cache = {}
last_results = None


def _build():
    from concourse import bacc, tile, mybir
    from concourse.masks import make_identity

    f32 = mybir.dt.float32
    f32r = mybir.dt.float32r
    bf16 = mybir.dt.bfloat16
    fp16 = mybir.dt.float16
    Alu = mybir.AluOpType
    Act = mybir.ActivationFunctionType
    PSUM = tile.bass.MemorySpace.PSUM

    nc = bacc.Bacc("TRN2", target_bir_lowering=False, debug=False,
                   num_devices=N_CORES)

    xs_d = nc.dram_tensor("xs", [N_LOC, C, HW], fp16, kind="ExternalInput")
    gft_d = nc.dram_tensor("gft", [D1, NM], f32r, kind="ExternalInput")
    wt_d = nc.dram_tensor("wt", [D1, D], f32r, kind="ExternalInput")
    out_d = nc.dram_tensor("out", [N_LOC, C, HW], f32, kind="ExternalOutput")

    with tile.TileContext(nc) as tc:
        with tc.tile_pool(name="const", bufs=1) as const:
            ident = const.tile([P, P], f32, tag="ident")
            make_identity(nc, ident[:, :])
            identr = const.tile([P, P], f32r, tag="identr")
            nc.vector.tensor_copy(identr[:, :], ident[:, :])
            identb = const.tile([P, P], fp16, tag="identb")
            nc.vector.tensor_copy(identb[:, :], ident[:, :])

            K_sb = const.tile([NM, C], f32r, tag="K_sb")
            V_sb = const.tile([NM, C], bf16, tag="V_sb")
            # per-item V rows at partition 0 (engine APs can't start at
            # partition 8/16/24); item 0 reads V_sb[0:8] directly
            V_n = [const.tile([M, C], bf16, tag=f"V{n}", name=f"V{n}")
                   for n in range(1, N_LOC)]
            KT = [const.tile([P, NM], fp16, tag=f"KT{kc}", name=f"KT{kc}")
                  for kc in range(KC)]

            with tc.tile_pool(name="wtp", bufs=1) as wtp, \
                 tc.tile_pool(name="psum0", bufs=1, space=PSUM) as psum0:
                wt_sb = []
                gft_sb = []
                for i in range(7):
                    rows = P if i < 6 else 1
                    g = wtp.tile([rows, NM], f32r, tag=f"gft{i}",
                                 name=f"gft{i}")
                    nc.sync.dma_start(g[:, :],
                                      gft_d.ap()[i * P:i * P + rows, :])
                    gft_sb.append(g)
                for i in range(7):
                    rows = P if i < 6 else 1
                    w = wtp.tile([rows, D], f32r, tag=f"wt{i}", name=f"wt{i}")
                    nc.sync.dma_start(w[:, :], wt_d.ap()[i * P:i * P + rows, :])
                    wt_sb.append(w)
                kvK = psum0.tile([NM, C], f32, tag="kvK")
                kvV = psum0.tile([NM, C], f32, tag="kvV")
                for i in range(7):
                    nc.tensor.matmul(
                        kvK[:, :], gft_sb[i][:, :], wt_sb[i][:, :C],
                        start=(i == 0), stop=(i == 6))
                for i in range(7):
                    nc.tensor.matmul(
                        kvV[:, :], gft_sb[i][:, :], wt_sb[i][:, C:],
                        start=(i == 0), stop=(i == 6))
                nc.vector.tensor_scalar(K_sb[:, :], kvK[:, :], 0.0, 6.0,
                                        op0=Alu.max, op1=Alu.min)
                nc.vector.tensor_scalar(V_sb[:, :], kvV[:, :], 0.0, 6.0,
                                        op0=Alu.max, op1=Alu.min)
                for n in range(1, N_LOC):
                    nc.sync.dma_start(V_n[n - 1][:, :],
                                      V_sb[n * M:(n + 1) * M, :])
                for kc in range(KC):
                    ktp = psum0.tile([P, NM], f32r, tag="ktp")
                    nc.tensor.transpose(ktp[:, :],
                                        K_sb[:, kc * P:(kc + 1) * P],
                                        identr[:NM, :NM])
                    nc.scalar.copy(KT[kc][:, :], ktp[:, :])

            with (
                tc.tile_pool(name="xp", bufs=12) as xp,
                tc.tile_pool(name="scf", bufs=2) as scfp,
                tc.tile_pool(name="abp", bufs=2) as abp,
                tc.tile_pool(name="sm", bufs=4) as sm,
                tc.tile_pool(name="aTp", bufs=2) as aTpool,
                tc.tile_pool(name="op", bufs=2) as op,
                tc.tile_pool(name="p32", bufs=1, space=PSUM) as p32,
                tc.tile_pool(name="ps_o", bufs=5, space=PSUM) as ps_o,
            ):
                def load_x(n):
                    xc = []
                    for kc in range(KC):
                        t = xp.tile([P, HW], fp16, tag="x", name="x")
                        nc.sync.dma_start(
                            t[:, :], xs_d.ap()[n, kc * P:(kc + 1) * P, :])
                        xc.append(t)
                    return xc

                def gen_out(n, aT, xc):
                    # mm2 + residual + store for item n; one step per yield
                    # so it interleaves with the next item's attention
                    # phase. Residual is split: t7<4 adds psum+x on DVE;
                    # t7>=4 accumulates x into psum on the PE (identity
                    # stationary) and ACT drains the finished tile.
                    for kc in range(KC):
                        osb = op.tile([P, HW], f32, tag="o", name="osb")
                        for t7 in range(NT5):
                            sl = slice(t7 * T5, (t7 + 1) * T5)
                            po = ps_o.tile([P, T5], f32, tag="po", name="po")
                            pe_res = t7 >= 4
                            nc.tensor.matmul(
                                po[:, :],
                                (V_sb[0:M, kc * P:(kc + 1) * P] if n == 0
                                 else V_n[n - 1][:, kc * P:(kc + 1) * P]),
                                aT[0:M, sl],
                                start=True, stop=not pe_res)
                            if pe_res:
                                nc.tensor.matmul(
                                    po[:, :], identb[:, :], xc[kc][:, sl],
                                    start=False, stop=True)
                                nc.scalar.copy(osb[:, sl], po[:, :])
                            else:
                                nc.vector.tensor_add(
                                    osb[:, sl], po[:, :],
                                    xc[kc][:, sl])
                            yield
                        nc.scalar.dma_start(
                            out_d.ap()[n, kc * P:(kc + 1) * P, :],
                            osb[:, :])
                        yield

                def drain(gen, steps):
                    if gen is None:
                        return None
                    try:
                        for _ in range(steps):
                            next(gen)
                    except StopIteration:
                        return None
                    return gen

                outgen = None
                xtiles = [None] * N_LOC
                xtiles[0] = load_x(0)
                xtiles[1] = load_x(1)
                xtiles[2] = load_x(2)
                for n in range(N_LOC):
                    xc = xtiles[n]

                    # mm1: scoresT [32, hw] with kc as the OUTER loop so
                    # the first matmul sweep starts as soon as x chunk 0
                    # lands. The 7 hw-subtiles pack 3-per-psum-bank on
                    # partition thirds (bf16 matmuls may target base
                    # partition 0/32/64; quadrant 3 is unusable).
                    SC = scfp.tile([NM, HW], f32, tag="scf")
                    pst = [p32.tile([P, T5], f32, tag=f"pst{b}",
                                    name=f"pst{b}") for b in range(3)]
                    for kc in range(KC):
                        for t5 in range(NT5):
                            q = t5 % 3
                            nc.tensor.matmul(
                                pst[t5 // 3][q * NM:(q + 1) * NM, :],
                                KT[kc][:, :],
                                xc[kc][:, t5 * T5:(t5 + 1) * T5],
                                start=(kc == 0), stop=(kc == KC - 1))
                            outgen = drain(outgen, 1)
                    for t5 in range(NT5):
                        q = t5 % 3
                        nc.vector.transpose(
                            SC[:, t5 * T5:(t5 + 1) * T5],
                            pst[t5 // 3][q * NM:(q + 1) * NM, :])
                        outgen = drain(outgen, 1)

                    # softmax along free dim in the folded layout:
                    # SC[p, 32j+c] = scores[q=32j+p, kvrow=c]; item n's
                    # scores live at c = n*8..n*8+8. attn is written back
                    # redirected to c = 0..8 so the unfold lands attnT on
                    # partitions 0-7.
                    sc3 = SC[:, :].rearrange("p (j c) -> p j c", c=32)
                    ssl = sc3[:, :, n * M:(n + 1) * M]
                    nmx = sm.tile([NM, NJ], f32, tag="nmx")
                    nc.vector.tensor_reduce(nmx[:, :], ssl,
                                            axis=mybir.AxisListType.X,
                                            op=Alu.max, negate=True)
                    nmx_b = nmx[:, :].unsqueeze(-1).broadcast_to([NM, NJ, M])
                    nc.gpsimd.tensor_add(ssl, ssl, nmx_b)
                    outgen = drain(outgen, 2)
                    nc.scalar.activation(ssl, ssl, Act.Exp)
                    den = sm.tile([NM, NJ], f32, tag="den")
                    nc.vector.tensor_reduce(den[:, :], ssl,
                                            axis=mybir.AxisListType.X,
                                            op=Alu.add)
                    r = sm.tile([NM, NJ], f32, tag="r")
                    nc.vector.reciprocal(r[:, :], den[:, :])
                    r_b = r[:, :].unsqueeze(-1).broadcast_to([NM, NJ, M])
                    # attn lands in bf16 (2-byte transposes; no f32r
                    # rounding rule), redirected to c 0..8. Cols 8..31 of
                    # each block stay unwritten; the unfold moves those
                    # bits into rows 8..31 of aT, which nothing reads.
                    AB = abp.tile([NM, HW], bf16, tag="ab")
                    ab3 = AB[:, :].rearrange("p (j c) -> p j c", c=32)
                    nc.vector.tensor_mul(ab3[:, :, 0:M], ssl, r_b)
                    outgen = drain(outgen, 2)

                    # unfold: attnT [8, hw] on partitions 0-7
                    aT = aTpool.tile([NM, HW], bf16, tag="aT")
                    hh = HW // 2
                    nc.vector.transpose(aT[:, :hh], AB[:, :hh])
                    nc.vector.transpose(aT[:, hh:], AB[:, hh:])

                    # flush the previous item's output phase, then queue ours
                    while outgen is not None:
                        outgen = drain(outgen, 4)
                    outgen = gen_out(n, aT, xc)
                    if n + 3 < N_LOC:
                        xtiles[n + 3] = load_x(n + 3)
                while outgen is not None:
                    outgen = drain(outgen, 4)

    nc.compile()
    return nc


def get_nc():
    if "nc" not in _cache:
        _cache["nc"] = _build()
    return _cache["nc"]


def make_in_maps(x, global_feature, W_kv, b_kv):
    x = np.ascontiguousarray(
        np.asarray(x, np.float32).reshape(N, C, HW).astype(np.float16))
    wt = np.zeros((D1, D), np.float32)
    wt[:D] = np.asarray(W_kv, np.float32).T
    wt[D] = np.asarray(b_kv, np.float32)
    gf = np.asarray(global_feature, np.float32)
    in_maps = []
    for i in range(N_CORES):
        gfl = gf[i * N_LOC:(i + 1) * N_LOC].reshape(NM, D)
        gft = np.zeros((D1, NM), np.float32)
        gft[:D] = gfl.T
        gft[D] = 1.0
        in_maps.append({
            "xs": np.ascontiguousarray(x[i * N_LOC:(i + 1) * N_LOC]),
            "gft": gft,
            "wt": wt,
        })
    return in_maps


def kernel(x, global_feature, W_kv, b_kv, trace=False):
    global last_results
    from concourse.bass_utils import run_bass_kernel_spmd

    nc = get_nc()
    in_maps = make_in_maps(x, global_feature, W_kv, b_kv)
    res = run_bass_kernel_spmd(nc, in_maps, core_ids=list(range(N_CORES)),
                               trace=trace)
    last_results = res
    out = np.concatenate([res.results[i]["out"][None] for i in range(N_CORES)],
                         axis=0)
    return out.reshape(N, C, H, W).astype(np.float32)


# revision 21
# speedup vs baseline: 1.0652x; 1.0522x over previous
"""Trainium2 Bass kernel for nn_Former_Mobile (mobile-former style cross-attention).

Computation (per batch item n):
    kv   = relu6(global_feature @ W_kv^T + b_kv)        # [m=8, 2c]
    K, V = kv[:, :c], kv[:, c:]                         # [8, c=384]
    q    = x reshaped [hw=3136, c]
    attn = softmax(q @ K^T)                             # [hw, 8]
    out  = (attn @ V) reshaped back + x                 # [c, hw]

Sharding: data-parallel over batch n across 8 NeuronCores (4 items each);
W_kv/b_kv replicated (bias folded in as a K=1 contraction row host-side).

The kernel is HBM-bound (~41MB in+out per core); engine placement:
  PE    : kv projection (one [32,768] series), mm1 scoresT (KT stationary
          M=32, x streaming, f32r, kc-outer so compute starts on the first
          x chunk; 7 subtiles packed 4-per-psum-bank on partition
          quarters), mm2 outT (V stationary K=8, attnT bf16 streaming),
          identity-matmul residual accumulate for part of the tiles.
  DVE   : 32x32 stream-transposes fold scoresT psum tiles into
          SC[p, 32j+c] = scores[q=32j+p, kvrow=c]; softmax along the free
          dim on c-slice n*8..n*8+8 writes bf16 attn redirected to c 0..8;
          a second stream-transpose yields attnT[8, hw] bf16 on partitions
          0-7 for mm2. Plus residual adds for t7<4.
  ACT   : exp, psum drains for the PE-residual tiles, output dma issues.
  Pool  : softmax subtract.
  Sync  : input dma issues.
mm1 operands are float32r (1 col/cycle at N>=256, fp32 psum accumulation);
softmax math is fp32; attn/V are bf16 (post-softmax, well within the
error budget); residual/store are fp32.
"""

import sys

if "/opt/trn_rl_repo" not in sys.path:
    sys.path.insert(0, "/opt/trn_rl_repo")

import numpy as np

N, C, H, W = 32, 384, 56, 56
HW = H * W                      # 3136
M, D = 8, 768
N_CORES = 8
N_LOC = N // N_CORES            # 4 batch items per core
NM = N_LOC * M                  # 32 kv rows per core
D1 = D + 1                      # 768 + bias row
KC = C // 128                   # 3 contraction chunks over c
P = 128
T5 = 448                        # mm1/mm2 free-dim tile (7 per hw row)
NT5 = HW // T5                  # 7
NJ = HW // 32                   # 98 folded blocks

_cache = {}
last_results = None


def _build():
    from concourse import bacc, tile, mybir
    from concourse.masks import make_identity

    f32 = mybir.dt.float32
    f32r = mybir.dt.float32r
    bf16 = mybir.dt.bfloat16
    fp16 = mybir.dt.float16
    Alu = mybir.AluOpType
    Act = mybir.ActivationFunctionType
    PSUM = tile.bass.MemorySpace.PSUM

    nc = bacc.Bacc("TRN2", target_bir_lowering=False, debug=False,
                   num_devices=N_CORES)

    xs_d = nc.dram_tensor("xs", [N_LOC, C, HW], fp16, kind="ExternalInput")
    gft_d = nc.dram_tensor("gft", [D1, NM], fp16, kind="ExternalInput")
    wt_d = nc.dram_tensor("wt", [D1, D], fp16, kind="ExternalInput")
    out_d = nc.dram_tensor("out", [N_LOC, C, HW], f32, kind="ExternalOutput")

    with tile.TileContext(nc) as tc:
        with tc.tile_pool(name="const", bufs=1) as const:
            ident = const.tile([P, P], f32, tag="ident")
            make_identity(nc, ident[:, :])
            identb = const.tile([P, P], fp16, tag="identb")
            nc.vector.tensor_copy(identb[:, :], ident[:, :])

            K_sb = const.tile([NM, C], fp16, tag="K_sb")
            V_sb = const.tile([NM, C], bf16, tag="V_sb")
            # per-item V rows at partition 0 (engine APs can't start at
            # partition 8/16/24); item 0 reads V_sb[0:8] directly
            V_n = [const.tile([M, C], bf16, tag=f"V{n}", name=f"V{n}")
                   for n in range(1, N_LOC)]
            KT = [const.tile([P, NM], fp16, tag=f"KT{kc}", name=f"KT{kc}")
                  for kc in range(KC)]

            with tc.tile_pool(name="wtp", bufs=1) as wtp, \
                 tc.tile_pool(name="psum0", bufs=1, space=PSUM) as psum0:
                wt_sb = []
                gft_sb = []
                for i in range(7):
                    rows = P if i < 6 else 1
                    g = wtp.tile([rows, NM], fp16, tag=f"gft{i}",
                                 name=f"gft{i}")
                    nc.sync.dma_start(g[:, :],
                                      gft_d.ap()[i * P:i * P + rows, :])
                    gft_sb.append(g)
                for i in range(7):
                    rows = P if i < 6 else 1
                    w = wtp.tile([rows, D], fp16, tag=f"wt{i}", name=f"wt{i}")
                    nc.sync.dma_start(w[:, :], wt_d.ap()[i * P:i * P + rows, :])
                    wt_sb.append(w)
                kvK = psum0.tile([NM, C], f32, tag="kvK")
                kvV = psum0.tile([NM, C], f32, tag="kvV")
                for i in range(7):
                    nc.tensor.matmul(
                        kvK[:, :], gft_sb[i][:, :], wt_sb[i][:, :C],
                        start=(i == 0), stop=(i == 6))
                for i in range(7):
                    nc.tensor.matmul(
                        kvV[:, :], gft_sb[i][:, :], wt_sb[i][:, C:],
                        start=(i == 0), stop=(i == 6))
                nc.vector.tensor_scalar(K_sb[:, :], kvK[:, :], 0.0, 6.0,
                                        op0=Alu.max, op1=Alu.min)
                nc.vector.tensor_scalar(V_sb[:, :], kvV[:, :], 0.0, 6.0,
                                        op0=Alu.max, op1=Alu.min)
                for n in range(1, N_LOC):
                    nc.sync.dma_start(V_n[n - 1][:, :],
                                      V_sb[n * M:(n + 1) * M, :])
                for kc in range(KC):
                    ktp = psum0.tile([P, NM], fp16, tag="ktp")
                    nc.tensor.transpose(ktp[:, :],
                                        K_sb[:, kc * P:(kc + 1) * P],
                                        identb[:NM, :NM])
                    nc.scalar.copy(KT[kc][:, :], ktp[:, :])

            with (
                tc.tile_pool(name="xp", bufs=12) as xp,
                tc.tile_pool(name="scf", bufs=2) as scfp,
                tc.tile_pool(name="abp", bufs=2) as abp,
                tc.tile_pool(name="sm", bufs=4) as sm,
                tc.tile_pool(name="aTp", bufs=2) as aTpool,
                tc.tile_pool(name="op", bufs=2) as op,
                tc.tile_pool(name="p32", bufs=1, space=PSUM) as p32,
                tc.tile_pool(name="ps_o", bufs=5, space=PSUM) as ps_o,
            ):
                def load_x(n):
                    xc = []
                    for kc in range(KC):
                        t = xp.tile([P, HW], fp16, tag="x", name="x")
                        nc.sync.dma_start(
                            t[:, :], xs_d.ap()[n, kc * P:(kc + 1) * P, :])
                        xc.append(t)
                    return xc

                def gen_out(n, aT, xc):
                    # mm2 + residual + store for item n; one step per yield
                    # so it interleaves with the next item's attention
                    # phase. Residual is split: t7<4 adds psum+x on DVE;
                    # t7>=4 accumulates x into psum on the PE (identity
                    # stationary) and ACT drains the finished tile.
                    for kc in range(KC):
                        osb = op.tile([P, HW], f32, tag="o", name="osb")
                        for t7 in range(NT5):
                            sl = slice(t7 * T5, (t7 + 1) * T5)
                            po = ps_o.tile([P, T5], f32, tag="po", name="po")
                            pe_res = t7 >= 4
                            nc.tensor.matmul(
                                po[:, :],
                                (V_sb[0:M, kc * P:(kc + 1) * P] if n == 0
                                 else V_n[n - 1][:, kc * P:(kc + 1) * P]),
                                aT[0:M, sl],
                                start=True, stop=not pe_res)
                            if pe_res:
                                nc.tensor.matmul(
                                    po[:, :], identb[:, :], xc[kc][:, sl],
                                    start=False, stop=True)
                                nc.scalar.copy(osb[:, sl], po[:, :])
                            else:
                                nc.vector.tensor_add(
                                    osb[:, sl], po[:, :],
                                    xc[kc][:, sl])
                            yield
                        nc.scalar.dma_start(
                            out_d.ap()[n, kc * P:(kc + 1) * P, :],
                            osb[:, :])
                        yield

                def drain(gen, steps):
                    if gen is None:
                        return None
                    try:
                        for _ in range(steps):
                            next(gen)
                    except StopIteration:
                        return None
                    return gen

                outgen = None
                xtiles = [None] * N_LOC
                xtiles[0] = load_x(0)
                xtiles[1] = load_x(1)
                xtiles[2] = load_x(2)
                for n in range(N_LOC):
                    xc = xtiles[n]

                    # mm1: scoresT [32, hw] with kc as the OUTER loop so
                    # the first matmul sweep starts as soon as x chunk 0
                    # lands. The 7 hw-subtiles pack 3-per-psum-bank on
                    # partition thirds (bf16 matmuls may target base
                    # partition 0/32/64; quadrant 3 is unusable).
                    SC = scfp.tile([NM, HW], f32, tag="scf")
                    pst = [p32.tile([P, T5], f32, tag=f"pst{b}",
                                    name=f"pst{b}") for b in range(3)]
                    for kc in range(KC):
                        for t5 in range(NT5):
                            q = t5 % 3
                            nc.tensor.matmul(
                                pst[t5 // 3][q * NM:(q + 1) * NM, :],
                                KT[kc][:, :],
                                xc[kc][:, t5 * T5:(t5 + 1) * T5],
                                start=(kc == 0), stop=(kc == KC - 1))
                            outgen = drain(outgen, 1)
                    for t5 in range(NT5):
                        q = t5 % 3
                        nc.vector.transpose(
                            SC[:, t5 * T5:(t5 + 1) * T5],
                            pst[t5 // 3][q * NM:(q + 1) * NM, :])
                        outgen = drain(outgen, 1)

                    # softmax along free dim in the folded layout:
                    # SC[p, 32j+c] = scores[q=32j+p, kvrow=c]; item n's
                    # scores live at c = n*8..n*8+8. attn is written back
                    # redirected to c = 0..8 so the unfold lands attnT on
                    # partitions 0-7.
                    sc3 = SC[:, :].rearrange("p (j c) -> p j c", c=32)
                    ssl = sc3[:, :, n * M:(n + 1) * M]
                    nmx = sm.tile([NM, NJ], f32, tag="nmx")
                    nc.vector.tensor_reduce(nmx[:, :], ssl,
                                            axis=mybir.AxisListType.X,
                                            op=Alu.max, negate=True)
                    nmx_b = nmx[:, :].unsqueeze(-1).broadcast_to([NM, NJ, M])
                    nc.gpsimd.tensor_add(ssl, ssl, nmx_b)
                    outgen = drain(outgen, 2)
                    nc.scalar.activation(ssl, ssl, Act.Exp)
                    den = sm.tile([NM, NJ], f32, tag="den")
                    nc.vector.tensor_reduce(den[:, :], ssl,
                                            axis=mybir.AxisListType.X,
                                            op=Alu.add)
                    r = sm.tile([NM, NJ], f32, tag="r")
                    nc.vector.reciprocal(r[:, :], den[:, :])
                    r_b = r[:, :].unsqueeze(-1).broadcast_to([NM, NJ, M])
                    # attn lands in bf16 (2-byte transposes; no f32r
                    # rounding rule), redirected to c 0..8. Cols 8..31 of
                    # each block stay unwritten; the unfold moves those
                    # bits into rows 8..31 of aT, which nothing reads.
                    AB = abp.tile([NM, HW], bf16, tag="ab")
                    ab3 = AB[:, :].rearrange("p (j c) -> p j c", c=32)
                    nc.vector.tensor_mul(ab3[:, :, 0:M], ssl, r_b)
                    outgen = drain(outgen, 2)

                    # unfold: attnT [8, hw] on partitions 0-7
                    aT = aTpool.tile([NM, HW], bf16, tag="aT")
                    hh = HW // 2
                    nc.vector.transpose(aT[:, :hh], AB[:, :hh])
                    nc.vector.transpose(aT[:, hh:], AB[:, hh:])

                    # flush the previous item's output phase, then queue ours
                    while outgen is not None:
                        outgen = drain(outgen, 4)
                    outgen = gen_out(n, aT, xc)
                    if n + 3 < N_LOC:
                        xtiles[n + 3] = load_x(n + 3)
                while outgen is not None:
                    outgen = drain(outgen, 4)

    nc.compile()
    return nc


def get_nc():
    if "nc" not in _cache:
        _cache["nc"] = _build()
    return _cache["nc"]


def make_in_maps(x, global_feature, W_kv, b_kv):
    x = np.ascontiguousarray(
        np.asarray(x, np.float32).reshape(N, C, HW).astype(np.float16))
    wt = np.zeros((D1, D), np.float16)
    wt[:D] = np.asarray(W_kv, np.float32).T
    wt[D] = np.asarray(b_kv, np.float32)
    gf = np.asarray(global_feature, np.float32)
    in_maps = []
    for i in range(N_CORES):
        gfl = gf[i * N_LOC:(i + 1) * N_LOC].reshape(NM, D)
        gft = np.zeros((D1, NM), np.float16)
        gft[:D] = gfl.T
        gft[D] = 1.0
        in_maps.append({
            "xs": np.ascontiguousarray(x[i * N_LOC:(i + 1) * N_LOC]),
            "gft": gft,
            "wt": wt,
        })
    return in_maps


def kernel(x, global_feature, W_kv, b_kv, trace=False):
    global last_results
    from concourse.bass_utils import run_bass_kernel_spmd

    nc = get_nc()
    in_maps = make_in_maps(x, global_feature, W_kv, b_kv)
    res = run_bass_kernel_spmd(nc, in_maps, core_ids=list(range(N_CORES)),
                               trace=trace)
    last_results = res
    out = np.concatenate([res.results[i]["out"][None] for i in range(N_CORES)],
                         axis=0)
    return out.reshape(N, C, H, W).astype(np.float32)
